# revision 2
# baseline (speedup 1.0000x reference)
"""Trainium2 Bass kernel for dynamic-conv1d attention-scale module.

Computes out = x + x * scale where
  scale[b,c,h,w] = sum_k attn[b,k,h,w] * w_sum[k,c]
  attn = softmax_k(logits/T),  logits[b,k,h,w] = fc2 @ relu(fc1 * qm)
  w_sum = weight.sum(axis=1)

Device strategy (8 NeuronCores, data-parallel over batch x H-halves):
  * quality_map >= 0 and fc1 is a bias-free 1x1 conv =>
    relu(fc1_w * q) == q * relu(fc1_w), so logits[k] = g[k]*q + b2[k]
    with g = fc2_w @ relu(fc1_w) folded on host and baked into the
    program as tensor_scalar immediates (the program is compiled per
    call, so weights-derived scalars are free and need no DMA).
  * softmax rows sum to 1 => 1 + scale = sum_k attn_k * (w_sum[k,c]+1),
    computed by one contract-12 f32r matmul per tile at ~fp32 accuracy
    via the 3-term compensated product hi@w1_hi + lo@w1_hi + hi@w1_lo
    (attn = hi + lo split on-device at the f32r grid, w1 split on host
    at the bf16 grid; bf16 is exactly representable in f32r).
  * Prologue is latency-tuned: q rides first on the sync queue, logits
    run as immediate tensor_scalars on the otherwise-idle DVE while the
    ACT LUT preloads (triggered by a dummy exp), one wide Exp, one
    broadcast normalize multiply, then the f32r hi/lo split.  A tiny
    warm-up DMA absorbs the scalar queue startup latency so the
    [hi|lo](+hi) dump (4 partition-blocks of big contiguous
    per-partition descriptors) starts promptly; per-chunk transposing
    gathers on the gpsimd queue build the k-major [12, chunk] rhs,
    each gating only on the dump blocks it reads.
  * x and y stream through HBM as bf16 (host converts both ways),
    halving the dominant traffic vs fp32.  All 18 x-tile loads are
    issued upfront on the sync queue (x is fully SBUF-resident,
    9.4 MB), so the input stream saturates HBM from t=0 regardless of
    compute and frees bandwidth for y later.
  * (1+scale) -> output multiply: fill-phase tiles use a direct DVE
    multiply from PSUM (DVE is idle while the ACT chain ramps), later
    tiles an ACT fp32->bf16 copy + packed 2x bf16 DVE multiply, which
    keeps the ACT and DVE makespans equal.  y DMAs are issued on the
    scalar ring lagged by 2 tiles so their sequencer-side waits are
    always already satisfied and never stall the copy stream.
"""

import sys

if "/opt/trn_rl_repo" not in sys.path:
    sys.path.insert(0, "/opt/trn_rl_repo")

import ml_dtypes
import numpy as np

import concourse.bacc as bacc
import concourse.mybir as mybir
from concourse.bass_utils import run_bass_kernel_spmd
from concourse.tile import TileContext

_B, _C, _H, _W = 4, 256, 192, 192
_K = 4
_TEMP = 34.0
_NCORES = 8
_HS = _H // 2            # 96 rows of H per shard
_N = _HS * _W            # 18432 pixels per core
_P = 128                 # SBUF partitions
_AP = 128                # partitions for attention pointwise math
_AF = _N // _AP          # 144 pixels per partition
_CH = 2048               # pixels per main-loop tile
_NT = _N // _CH          # 9 chunks
_BIG = 2 * _CH           # x/y DMA tile width (8 KB descriptors)
_NPAIR = (_NT + 1) // 2  # 5 x/y DMA tiles per channel-half (last is half)
_MM = 512                # matmul moving free dim (one PSUM bank)
_KF = _K * _AF           # cols per hi/lo set in the [128, .] layout
_DT = mybir.dt.float32
_DTR = mybir.dt.float32r
_BF = mybir.dt.bfloat16


def _build_nc(gscale, gbias):
    nc = bacc.Bacc()
    x_d = nc.dram_tensor("x", [_C, _N], _BF, kind="ExternalInput")
    qm_d = nc.dram_tensor("qm", [_AP, _AF], _DT, kind="ExternalInput")
    w_d = nc.dram_tensor("w", [3 * _K, _C], _DTR, kind="ExternalInput")
    y_d = nc.dram_tensor("y", [_C, _N], _BF, kind="ExternalOutput")
    # Plain per-partition dump of [hi | lo | hi] attention rows: the
    # transposing gather happens on the read side.
    rows_s = nc.dram_tensor("rows_scratch", [_AP, 3 * _KF], _DTR)
    # Pixel-major [12, 128, 144] view of the dump for the chunked gathers.
    rows_v = rows_s.rearrange("p (s f) -> s p f", s=3 * _K)

    with TileContext(nc) as tc:
        with (
            tc.tile_pool(name="const", bufs=1) as cpool,
            tc.tile_pool(name="attn", bufs=1) as apool,
            tc.tile_pool(name="rowring", bufs=6) as rpool,
            tc.tile_pool(name="xin", bufs=2 * _NT) as xpool,
            tc.tile_pool(name="sc", bufs=3) as spool,
            tc.tile_pool(name="yout", bufs=6) as ypool,
            tc.tile_pool(name="ps", bufs=2, space="PSUM") as pspool,
        ):
            # Dummy Exp on a zero tile: pulls the ACT LUT load off the
            # critical path.
            dum = apool.tile([1, 2], _DT)
            nc.vector.memzero(dum[:, :])
            nc.scalar.activation(
                out=dum[:, 1:2],
                in_=dum[:, 0:1],
                func=mybir.ActivationFunctionType.Exp,
            )
            # Warm up the scalar hardware DGE queue so the rows dump's
            # first descriptor doesn't pay the queue startup latency.
            warm = apool.tile([1, 8], _DT)
            nc.scalar.dma_start(out=warm[:, :], in_=qm_d[0:1, 0:8])
            # q rides first on the sync queue so it lands with minimal
            # latency; then the full upfront x stream.
            q = apool.tile([_AP, _AF], _DT)
            wt = cpool.tile([3 * _K, _C], _DTR)   # [w1_hi; w1_hi; w1_lo]
            nc.sync.dma_start(out=q[:, :], in_=qm_d[:, :])
            nc.sync.dma_start(out=wt[:, :], in_=w_d[:, :])
            xts = []
            for t in range(_NT):
                for ch in range(_C // _P):
                    xt = xpool.tile([_P, _CH], _BF)
                    nc.sync.dma_start(
                        out=xt[:, :],
                        in_=x_d[ch * _P : (ch + 1) * _P, t * _CH : (t + 1) * _CH],
                    )
                    xts.append(xt)

            # ---- attention pointwise in [128, 144] layout ----
            # logits on the DVE with immediate g/b (no weight DMA), one
            # wide Exp on ACT once the LUT is resident.
            lg = apool.tile([_AP, _KF], _DT)
            for k in range(_K):
                nc.vector.tensor_scalar(
                    out=lg[:, k * _AF : (k + 1) * _AF],
                    in0=q[:, :],
                    scalar1=float(gscale[k]),
                    scalar2=float(gbias[k]),
                    op0=mybir.AluOpType.mult,
                    op1=mybir.AluOpType.add,
                )
            e = apool.tile([_AP, _KF], _DT)
            nc.scalar.activation(
                out=e[:, :],
                in_=lg[:, :],
                func=mybir.ActivationFunctionType.Exp,
            )
            d0 = apool.tile([_AP, _AF], _DT)
            d1 = apool.tile([_AP, _AF], _DT)
            nc.vector.tensor_add(
                out=d0[:, :], in0=e[:, 0:_AF], in1=e[:, _AF : 2 * _AF]
            )
            nc.vector.tensor_add(
                out=d1[:, :], in0=e[:, 2 * _AF : 3 * _AF], in1=e[:, 3 * _AF :]
            )
            nc.vector.tensor_add(out=d0[:, :], in0=d0[:, :], in1=d1[:, :])
            r = apool.tile([_AP, _AF], _DT)
            nc.vector.reciprocal_approx_accurate(
                out=r[:, :], in_=d0[:, :], scratch=d1[:, :]
            )
            # attn = e * (1/d) in one broadcast multiply (in place, fp32)
            nc.vector.tensor_mul(
                out=e[:, :].rearrange("p (k i) -> p k i", k=_K),
                in0=e[:, :].rearrange("p (k i) -> p k i", k=_K),
                in1=r[:, :].rearrange("p i -> p () i").broadcast_to(
                    [_AP, _K, _AF]
                ),
            )
            # Split at the f32r grid: attn = hi + lo
            ahl = apool.tile([_AP, 2 * _KF], _DTR)  # [hi | lo]
            nc.vector.tensor_copy(out=ahl[:, 0:_KF], in_=e[:, :])
            nc.vector.tensor_sub(
                out=ahl[:, _KF : 2 * _KF],
                in0=e[:, :],
                in1=ahl[:, 0:_KF].bitcast(_DT),
            )
            # Straight dumps (no rearrange): [hi | lo] then the hi repeat,
            # in 4 partition-blocks so early chunks' gathers gate only on
            # the early blocks.
            for pb in range(0, _AP, 32):
                nc.scalar.dma_start(
                    out=rows_s[pb : pb + 32, 0 : 2 * _KF],
                    in_=ahl[pb : pb + 32, :],
                )
                nc.scalar.dma_start(
                    out=rows_s[pb : pb + 32, 2 * _KF : 3 * _KF],
                    in_=ahl[pb : pb + 32, 0:_KF],
                )

            # ---- main stream: out = x * (1 + scale) ----
            pending = []
            for t in range(_NT):
                nsl = slice(t * _CH, (t + 1) * _CH)
                rt = rpool.tile([3 * _K, _CH], _DTR)
                a, b = t * _CH, (t + 1) * _CH
                p_lo, f_lo = divmod(a, _AF)
                p_hi, f_hi = divmod(b, _AF)
                off = 0
                if f_lo:
                    ln = _AF - f_lo
                    nc.gpsimd.dma_start(
                        out=rt[:, 0:ln].rearrange("s (p f) -> s p f", p=1),
                        in_=rows_v[:, p_lo : p_lo + 1, f_lo:_AF],
                    )
                    off = ln
                    p_lo += 1
                npm = p_hi - p_lo
                nc.gpsimd.dma_start(
                    out=rt[:, off : off + npm * _AF].rearrange(
                        "s (p f) -> s p f", p=npm
                    ),
                    in_=rows_v[:, p_lo:p_hi, :],
                )
                if f_hi:
                    nc.gpsimd.dma_start(
                        out=rt[:, _CH - f_hi : _CH].rearrange(
                            "s (p f) -> s p f", p=1
                        ),
                        in_=rows_v[:, p_hi : p_hi + 1, 0:f_hi],
                    )
                for ch in range(_C // _P):
                    idx = t * 2 + ch
                    lhsT = wt[:, ch * _P : (ch + 1) * _P]
                    xt = xts[idx]
                    ps = pspool.tile([_P, _CH], _DT)
                    for j in range(_CH // _MM):
                        nc.tensor.matmul(
                            ps[:, j * _MM : (j + 1) * _MM],
                            lhsT,
                            rt[:, j * _MM : (j + 1) * _MM],
                            start=True,
                            stop=True,
                        )
                    ot = ypool.tile([_P, _CH], _BF)
                    if idx >= 7:
                        # split path: ACT converts PSUM->bf16, DVE runs the
                        # packed 2x bf16 multiply
                        st = spool.tile([_P, _CH], _BF)
                        nc.scalar.activation(
                            out=st[:, :],
                            in_=ps[:, :],
                            func=mybir.ActivationFunctionType.Copy,
                        )
                        nc.vector.tensor_mul(
                            out=ot[:, :], in0=xt[:, :], in1=st[:, :]
                        )
                    else:
                        # direct path (pipeline fill): DVE is idle while the
                        # ACT copy chain ramps; keeps ACT/DVE makespans equal
                        nc.vector.tensor_mul(
                            out=ot[:, :], in0=xt[:, :], in1=ps[:, :]
                        )
                    pending.append((ot, ch * _P, nsl))
                    if len(pending) > 2:
                        po, pc, pn = pending.pop(0)
                        nc.scalar.dma_start(
                            out=y_d[pc : pc + _P, pn], in_=po[:, :]
                        )
            for po, pc, pn in pending:
                nc.scalar.dma_start(out=y_d[pc : pc + _P, pn], in_=po[:, :])
    nc.compile()
    return nc


def _prepare_in_maps(x, quality_map, fc1_w, fc2_w, fc2_b, weight):
    x = np.asarray(x, dtype=np.float32)
    qm = np.asarray(quality_map, dtype=np.float32)
    fc1 = np.asarray(fc1_w, dtype=np.float32)
    fc2 = np.asarray(fc2_w, dtype=np.float32)
    b2 = np.asarray(fc2_b, dtype=np.float32)
    w = np.asarray(weight, dtype=np.float32)

    # Weight-only folding (host): g = fc2 @ relu(fc1); w1 = w_sum + 1,
    # split at the bf16 grid: w1 = w1_hi + w1_lo (w1_hi exact in f32r).
    g = (fc2 @ np.maximum(fc1[:, 0], 0.0)).astype(np.float32)  # [K]
    w1 = (w.sum(axis=1) + 1.0).astype(np.float32)              # [K, C]
    w1_hi = w1.astype(ml_dtypes.bfloat16).astype(np.float32)
    w1_lo = (w1 - w1_hi).astype(np.float32)
    wstack = np.concatenate([w1_hi, w1_hi, w1_lo], axis=0)     # [12, C]
    gscale = (g / _TEMP).astype(np.float32)
    gbias = (b2 / _TEMP).astype(np.float32)

    in_maps = []
    xb = x.astype(ml_dtypes.bfloat16)
    for core in range(_NCORES):
        b, half = divmod(core, 2)
        h0 = half * _HS
        xs = np.ascontiguousarray(xb[b, :, h0 : h0 + _HS, :]).reshape(_C, _N)
        qs = np.ascontiguousarray(qm[b, 0, h0 : h0 + _HS, :]).reshape(_AP, _AF)
        in_maps.append({"x": xs, "qm": qs, "w": wstack})
    return in_maps, gscale, gbias


def _run(in_maps, gscale, gbias, **kwargs):
    nc = _build_nc(gscale, gbias)
    return run_bass_kernel_spmd(nc, in_maps, core_ids=list(range(_NCORES)), **kwargs)


def kernel(x, quality_map, fc1_w, fc2_w, fc2_b, weight):
    in_maps, gscale, gbias = _prepare_in_maps(
        x, quality_map, fc1_w, fc2_w, fc2_b, weight
    )
    res = _run(in_maps, gscale, gbias)
    out = np.empty((_B, _C, _H, _W), dtype=np.float32)
    for core in range(_NCORES):
        b, half = divmod(core, 2)
        h0 = half * _HS
        ys = np.asarray(res.results[core]["y"]).astype(np.float32)
        out[b, :, h0 : h0 + _HS, :] = ys.reshape(_C, _HS, _W)
    return out
